# revision 1
# baseline (speedup 1.0000x reference)
"""Multi-head attention (B=4, S=2048, E=1024, H=16, D=64) on 8 TRN2 NeuronCores.

Sharding: data-parallel over batch (4) x tensor-parallel over heads (2).
Core c handles batch c//2 with heads [tp*8, tp*8+8), tp = c%2.

Per-core dataflow (all matmuls bf16 inputs, fp32 PSUM accumulation):
  phase 1: QKV projection.
    Q^T,K^T computed column-major ([head-dim, seq]) via lhsT=W, rhs=x^T.
    V computed row-major ([seq, head-dim]) via lhsT=x^T, rhs=Wv; a host-
    appended bias row on Wv plus an on-chip ones row implements +bias; an
    on-chip ones *column* appended to V makes the attention PV matmul also
    produce softmax row-sums.
  phase 2: per head: S^T = K^T-tiles @ Q^T (scores transposed, k on
    partitions), exp on ScalarE straight from PSUM (fused 1/8 scale, bf16
    out), O^T[65,2048] accumulated over 16 k-blocks where row 64 = softmax
    denominator l.  Normalize: partition-broadcast DMA of l, DVE reciprocal
    + multiply (fused bf16 cast).
  phase 3: out-proj partial y^T = Wo_shard^T-tiles @ O_n^T (+bias on tp0),
    ReduceScatter(add) over the TP pair, each rank keeps 512 rows of y^T.

Host: pre-transposes x, pre-slices/casts weights to bf16, and transposes
the gathered y^T shards back to [B,S,E] fp32.
"""

import numpy as np
import ml_dtypes

B, S, E, H, D = 4, 2048, 1024, 16, 64
NCORES = 8
TP = 2
HPC = H // TP          # heads per core = 8
PAIRS = HPC // 2       # head pairs per core = 4
P = 128
KT = E // P            # 8 contraction tiles over E
SQ = S // 512          # 4 sequence chunks of 512
SB = S // P            # 16 sequence blocks of 128
CS = HPC * D           # per-core qkv col shard width = 512

_BF16 = ml_dtypes.bfloat16

_cached = {}


def _split_drain_waits(nc, mybir, max_waits=1):
    """This walrus build rejects instructions carrying more than ~2 sem
    waits; hoist extras onto preceding same-engine nops."""
    for f in nc.m.functions:
        for bb in f.blocks:
            insts = bb.instructions
            i = 0
            while i < len(insts):
                inst = insts[i]
                si = inst.sync_info
                if si is not None and len(si.on_wait) > max_waits:
                    extra = list(si.on_wait[max_waits:])
                    keep = list(si.on_wait[:max_waits])
                    for j, w in enumerate(extra):
                        nop = mybir.InstNoOp(
                            name=f"{inst.name}-waitsplit{j}", ins=[], outs=[]
                        )
                        nop.engine = inst.engine
                        nop.sync_info = mybir.SyncInfo(on_wait=[w], on_update=[])
                        nc.register_instruction(nop)
                        insts.insert(i, nop)
                        i += 1
                    inst.sync_info = mybir.SyncInfo(
                        on_wait=keep, on_update=list(si.on_update)
                    )
                i += 1


def _build_program(collective=True):
    import concourse.bass as bass
    import concourse.tile as tile
    from concourse import mybir

    f32 = mybir.dt.float32
    bf16 = mybir.dt.bfloat16

    nc = bass.Bass("TRN2", num_devices=NCORES, debug=False)

    xt_d = nc.dram_tensor("xt", [E, S], bf16, kind="ExternalInput")
    wqk_d = nc.dram_tensor("wqk", [E, 2 * CS], bf16, kind="ExternalInput")
    bqk_d = nc.dram_tensor("bqk", [P, 2 * CS // P], f32, kind="ExternalInput")
    wv_d = nc.dram_tensor("wv", [E + 1, CS], bf16, kind="ExternalInput")
    wo_d = nc.dram_tensor("wo", [CS, E], bf16, kind="ExternalInput")
    bo_d = nc.dram_tensor("bo", [P, E // P], f32, kind="ExternalInput")
    import os as _os
    out_d = nc.dram_tensor("out", [E // TP, S], f32, kind="ExternalOutput")
    _taps = _os.environ.get("K_TAPS") == "1"
    if _taps:
        tap_qk = nc.dram_tensor("tap_qk", [P, S], bf16, kind="ExternalOutput")
        tap_v = nc.dram_tensor("tap_v", [P, HPC * (D + 1)], bf16, kind="ExternalOutput")
        tap_on = nc.dram_tensor("tap_on", [PAIRS * P, S], bf16, kind="ExternalOutput")

    groups = [[2 * i, 2 * i + 1] for i in range(NCORES // 2)]

    with tile.TileContext(nc) as tc:
        with (
            tc.tile_pool(name="const", bufs=1) as const,
            tc.tile_pool(name="win", bufs=1) as win,
            tc.tile_pool(name="qk", bufs=1) as qkp,
            tc.tile_pool(name="vsb", bufs=1) as vp,
            tc.tile_pool(name="on", bufs=1) as onp,
            tc.tile_pool(name="pt", bufs=6) as ptp,
            tc.tile_pool(name="rec", bufs=2) as recp,
            tc.tile_pool(name="ysb", bufs=4) as yp,
            tc.tile_pool(name="dram", bufs=1, space="DRAM") as dram,
        ):
          _repeat = int(_os.environ.get("K_REPEAT", "1"))
          for _rep in range(_repeat):
            # ---- constants / weights into SBUF ----
            bqk_sb = const.tile([P, 2 * CS // P], f32, tag="bqk")
            nc.sync.dma_start(out=bqk_sb[:], in_=bqk_d.ap())
            bo_sb = const.tile([P, E // P], f32, tag="bo")
            nc.sync.dma_start(out=bo_sb[:], in_=bo_d.ap())
            ones_sb = const.tile([1, S], bf16, tag="ones")
            nc.vector.memset(ones_sb[:], 1.0)

            wv_sb = [win.tile([P, CS], bf16, tag=f"wv{k}", name=f"wv{_rep}_{k}") for k in range(KT)]
            for k in range(KT):
                nc.sync.dma_start(out=wv_sb[k][:], in_=wv_d[k * P:(k + 1) * P, :])
            wvb_sb = win.tile([1, CS], bf16, tag="wvb")
            nc.sync.dma_start(out=wvb_sb[:], in_=wv_d[E:E + 1, :])

            xt_sb = [win.tile([P, S], bf16, tag=f"xt{k}", name=f"xt{_rep}_{k}") for k in range(KT)]
            for k in range(KT):
                nc.sync.dma_start(out=xt_sb[k][:], in_=xt_d[k * P:(k + 1) * P, :])

            wqk_sb = [win.tile([P, 2 * CS], bf16, tag=f"wqk{k}", name=f"wqk{_rep}_{k}") for k in range(KT)]
            for k in range(KT):
                nc.sync.dma_start(out=wqk_sb[k][:], in_=wqk_d[k * P:(k + 1) * P, :])

            wo_sb = [win.tile([P, E], bf16, tag=f"wo{p}", name=f"wo{_rep}_{p}") for p in range(PAIRS)]
            for p in range(PAIRS):
                nc.sync.dma_start(out=wo_sb[p][:], in_=wo_d[p * P:(p + 1) * P, :])

            # persistent activations
            qk_sb = [qkp.tile([P, S], bf16, tag=f"qk{c}", name=f"qk{_rep}_{c}") for c in range(2 * CS // P)]
            v_sb = [vp.tile([P, HPC, D + 1], bf16, tag=f"v{s}", name=f"v{_rep}_{s}") for s in range(SB)]
            on_sb = [onp.tile([P, S], bf16, tag=f"on{p}", name=f"on{_rep}_{p}") for p in range(PAIRS)]

            # ---- shared PSUM pools (8 banks static across all phases) ----
            work_cm = tc.tile_pool(name="work", bufs=2, space="PSUM")
            work = work_cm.__enter__()
            acc_cm = tc.tile_pool(name="acc", bufs=2, space="PSUM")
            acc = acc_cm.__enter__()

            def emit_v_phase():
                # V: out[seq-block, vcols] ; lhsT = x^T tile, rhs = Wv tile
                for s in range(SB):
                    pv = work.tile([P, CS], f32, tag="w", name=f"pv{_rep}_{s}")
                    for k in range(KT):
                        nc.tensor.matmul(
                            pv[:],
                            xt_sb[k][:, s * P:(s + 1) * P],
                            wv_sb[k][:],
                            start=(k == 0),
                            stop=False,
                        )
                    # bias row: ones row (K=1) x Wv bias row
                    nc.tensor.matmul(
                        pv[:],
                        ones_sb[:, s * P:(s + 1) * P],
                        wvb_sb[:],
                        start=False,
                        stop=True,
                    )
                    nc.vector.memset(v_sb[s][:, :, D:D + 1], 1.0)
                    nc.vector.tensor_copy(v_sb[s][:, :, 0:D], pv[:])

            def qk_chain(c, q, pool=None, tag="w"):
                def chain():
                    pq = (pool or work).tile([P, 512], f32, tag=tag, name=f"pq{_rep}_{c}_{q}")
                    for k in range(KT):
                        nc.tensor.matmul(
                            pq[:],
                            wqk_sb[k][:, c * P:(c + 1) * P],
                            xt_sb[k][:, q * 512:(q + 1) * 512],
                            start=(k == 0),
                            stop=(k == KT - 1),
                        )
                    nc.vector.tensor_scalar_add(
                        qk_sb[c][:, q * 512:(q + 1) * 512],
                        pq[:],
                        bqk_sb[:, c:c + 1],
                    )
                return chain

            def qk_pair_chains(p, pool=None, tag="w"):
                return [
                    qk_chain(c, q, pool, tag)
                    for c in (p, PAIRS + p)
                    for q in range(SQ)
                ]

            def emit_qk_pair(p, pool=None, tag="w"):
                for ch in qk_pair_chains(p, pool, tag):
                    ch()

            def emit_head(h, fillers=(), stride=2):
                fillers = list(fillers)
                p, half = h // 2, h % 2
                r0 = half * D
                qT = qk_sb[p]
                kT = qk_sb[PAIRS + p]
                po = [
                    acc.tile([D + 1, 1024], f32, tag="a", name=f"po{_rep}_{h}_{j}")
                    for j in range(2)
                ]
                for kb in range(SB):
                    if fillers and kb % stride == 1:
                        fillers.pop(0)()
                    pt = ptp.tile([P, S], bf16, tag="pt", name=f"pt{_rep}_{h}_{kb}")
                    for qh in range(2):
                        ps = work.tile([P, 1024], f32, tag="w", name=f"ps{_rep}_{h}_{kb}_{qh}")
                        for q2 in range(2):
                            q = 2 * qh + q2
                            nc.tensor.matmul(
                                ps[:, q2 * 512:(q2 + 1) * 512],
                                kT[r0:r0 + D, kb * P:(kb + 1) * P],
                                qT[r0:r0 + D, q * 512:(q + 1) * 512],
                                start=True,
                                stop=True,
                            )
                        nc.scalar.activation(
                            pt[:, qh * 1024:(qh + 1) * 1024],
                            ps[:],
                            mybir.ActivationFunctionType.Exp,
                            scale=0.125,
                        )
                        for q2 in range(2):
                            o = qh * 1024 + q2 * 512
                            nc.tensor.matmul(
                                po[qh][:, q2 * 512:(q2 + 1) * 512],
                                v_sb[kb][:, h, :],
                                pt[:, o:o + 512],
                                start=(kb == 0),
                                stop=(kb == SB - 1),
                            )
                # normalize: O^T[0:D] / l (l = row D), write bf16
                for qh in range(2):
                    lsb = recp.tile([1, 1024], f32, tag="lsb", name=f"lsb{_rep}_{h}_{qh}")
                    nc.vector.reciprocal(lsb[:], po[qh][D:D + 1, :])
                    lscr = dram.tile(
                        [1, 1024], f32, tag="lscr", name=f"lscr{_rep}_{h}_{qh}", bufs=2
                    )
                    nc.sync.dma_start(out=lscr[:], in_=lsb[:])
                    ldr = lscr[:]
                    lbc = bass.AP(
                        tensor=ldr.tensor,
                        offset=ldr.offset,
                        ap=[[0, D]] + [list(x) for x in ldr.ap[1:]],
                    )
                    rin = recp.tile([D, 1024], f32, tag="rin", name=f"rin{_rep}_{h}_{qh}")
                    nc.sync.dma_start(out=rin[:], in_=lbc)
                    nc.vector.tensor_mul(
                        on_sb[p][r0:r0 + D, qh * 1024:(qh + 1) * 1024],
                        po[qh][0:D, :],
                        rin[:],
                    )

            # interleave: QK pair 0 first so head 0's S-matmuls (and exp)
            # start ASAP; V is emitted after head 0 so its chains fill PE
            # whenever head 0 blocks (head 0's O-phase pulls v_sb[kb] just in
            # time).  Later QK pairs ride as per-kb fillers inside heads.
            _order = _os.environ.get("K_ORDER", "safe")
            if _order == "safe":
                emit_v_phase()
                emit_qk_pair(0)
                emit_head(0)
                emit_head(1)
                emit_qk_pair(1)
                emit_head(2)
                emit_head(3)
                emit_qk_pair(2)
                emit_head(4)
                emit_head(5)
                emit_qk_pair(3)
                emit_head(6)
                emit_head(7)
            elif _order == "fill2":
                emit_qk_pair(0)
                emit_head(0)
                emit_v_phase()
                emit_head(1, fillers=qk_pair_chains(1))
                emit_head(2)
                emit_head(3, fillers=qk_pair_chains(2))
                emit_head(4)
                emit_head(5, fillers=qk_pair_chains(3))
                emit_head(6)
                emit_head(7)
            elif _order == "burst":
                emit_qk_pair(0)
                emit_head(0)
                emit_v_phase()
                emit_head(1)
                emit_qk_pair(1)
                emit_head(2)
                emit_head(3)
                emit_qk_pair(2)
                emit_head(4)
                emit_head(5)
                emit_qk_pair(3)
                emit_head(6)
                emit_head(7)
            elif _order == "burst_acc":
                # QK bursts use the acc pool: po slots are free at pair
                # boundaries, so the burst hides under the exp lookahead
                emit_qk_pair(0)
                emit_head(0)
                emit_v_phase()
                emit_head(1)
                emit_qk_pair(1, acc, "a")
                emit_head(2)
                emit_head(3)
                emit_qk_pair(2, acc, "a")
                emit_head(4)
                emit_head(5)
                emit_qk_pair(3, acc, "a")
                emit_head(6)
                emit_head(7)
            elif _order == "fill4":
                emit_qk_pair(0)
                emit_head(0)
                emit_v_phase()
                c1, c2, c3 = qk_pair_chains(1), qk_pair_chains(2), qk_pair_chains(3)
                emit_head(1, fillers=c1[:4], stride=4)
                emit_head(2, fillers=c1[4:], stride=4)
                emit_head(3, fillers=c2[:4], stride=4)
                emit_head(4, fillers=c2[4:], stride=4)
                emit_head(5, fillers=c3[:4], stride=4)
                emit_head(6, fillers=c3[4:], stride=4)
                emit_head(7)

            # ---- phase 3: output projection + reduce-scatter ----
            # chunk-major layout: each sequence chunk is a contiguous block
            # (collective inputs must be contiguous)
            y_dram = dram.tile([SQ, E, 512], f32, tag="ydram")
            y_red = dram.tile([SQ, E // TP, 512], f32, tag="yred")
            for q in range(SQ):
                for e in range(E // P):
                    py = work.tile([P, 512], f32, tag="w", name=f"py{_rep}_{e}_{q}")
                    for p in range(PAIRS):
                        nc.tensor.matmul(
                            py[:],
                            wo_sb[p][:, e * P:(e + 1) * P],
                            on_sb[p][:, q * 512:(q + 1) * 512],
                            start=(p == 0),
                            stop=(p == PAIRS - 1),
                        )
                    ye = yp.tile([P, 512], f32, tag="ysb")
                    nc.vector.tensor_scalar_add(ye[:], py[:], bo_sb[:, e:e + 1])
                    nc.sync.dma_start(
                        out=y_dram[q, e * P:(e + 1) * P, :],
                        in_=ye[:],
                    )
                # reduce-scatter this sequence chunk while the next computes
                if collective:
                    nc.gpsimd.collective_compute(
                        "ReduceScatter",
                        mybir.AluOpType.add,
                        replica_groups=groups,
                        ins=[y_dram[q].opt()],
                        outs=[y_red[q].opt()],
                    )
                    nc.sync.dma_start(
                        out=out_d[:, q * 512:(q + 1) * 512],
                        in_=y_red[q],
                    )
                else:
                    nc.sync.dma_start(
                        out=out_d[:, q * 512:(q + 1) * 512],
                        in_=y_dram[q, 0:E // TP, :],
                    )

            if _taps:
                nc.sync.dma_start(out=tap_qk.ap(), in_=qk_sb[0][:])
                nc.sync.dma_start(
                    out=tap_v.ap(),
                    in_=v_sb[0].rearrange("p h d -> p (h d)"),
                )
                for _p in range(PAIRS):
                    nc.sync.dma_start(
                        out=tap_on[_p * P:(_p + 1) * P, :], in_=on_sb[_p][:]
                    )

            acc_cm.__exit__(None, None, None)
            work_cm.__exit__(None, None, None)

    _split_drain_waits(nc, mybir)
    return nc


def _host_shards(x, Wqkv, bqkv, Wo, bo):
    x = np.asarray(x, np.float32)
    Wqkv = np.asarray(Wqkv, np.float32)
    bqkv = np.asarray(bqkv, np.float32)
    Wo = np.asarray(Wo, np.float32)
    bo = np.asarray(bo, np.float32)

    in_maps = []
    for c in range(NCORES):
        b, tp = c // 2, c % 2
        lo = tp * CS
        xt = np.ascontiguousarray(x[b].T).astype(_BF16)
        wqk = np.concatenate(
            [Wqkv[:, lo:lo + CS], Wqkv[:, E + lo:E + lo + CS]], axis=1
        ).astype(_BF16)
        bqk = (
            np.concatenate([bqkv[lo:lo + CS], bqkv[E + lo:E + lo + CS]])
            .reshape(2 * CS // P, P)
            .T.astype(np.float32)
        )
        wv = np.concatenate(
            [Wqkv[:, 2 * E + lo:2 * E + lo + CS], bqkv[None, 2 * E + lo:2 * E + lo + CS]],
            axis=0,
        ).astype(_BF16)
        wo = Wo[lo:lo + CS, :].astype(_BF16)
        bo_c = bo if tp == 0 else np.zeros_like(bo)
        bo_c = bo_c.reshape(E // P, P).T.astype(np.float32)
        in_maps.append(
            {
                "xt": np.ascontiguousarray(xt),
                "wqk": np.ascontiguousarray(wqk),
                "bqk": np.ascontiguousarray(bqk),
                "wv": np.ascontiguousarray(wv),
                "wo": np.ascontiguousarray(wo),
                "bo": np.ascontiguousarray(bo_c),
            }
        )
    return in_maps


def _get_runner():
    """Build the Bass program once and wrap it in a cached 8-core jitted
    callable (same execution path run_bass_kernel_spmd uses under axon, but
    the XLA executable is reused across kernel() calls)."""
    if "runner" in _cached:
        return _cached["runner"]

    import jax
    from jax.sharding import Mesh, PartitionSpec, NamedSharding
    from jax.experimental.shard_map import shard_map
    from concourse import bass2jax, mybir

    nc = _build_program()
    _cached["nc"] = nc
    bass2jax.install_neuronx_cc_hook()

    partition_name = nc.partition_id_tensor.name if nc.partition_id_tensor else None
    in_names, out_names, out_avals = [], [], []
    for alloc in nc.m.functions[0].allocations:
        if not isinstance(alloc, mybir.MemoryLocationSet):
            continue
        name = alloc.memorylocations[0].name
        if alloc.kind == "ExternalInput":
            if name != partition_name:
                in_names.append(name)
        elif alloc.kind == "ExternalOutput":
            out_names.append(name)
            out_avals.append(
                jax.core.ShapedArray(tuple(alloc.tensor_shape), mybir.dt.np(alloc.dtype))
            )
    n_params = len(in_names)
    all_in_names = list(in_names) + list(out_names)
    if partition_name is not None:
        all_in_names.append(partition_name)

    def _body(*args):
        operands = list(args)
        if partition_name is not None:
            operands.append(bass2jax.partition_id_tensor())
        outs = bass2jax._bass_exec_p.bind(
            *operands,
            out_avals=tuple(out_avals),
            in_names=tuple(all_in_names),
            out_names=tuple(out_names),
            lowering_input_output_aliases=(),
            sim_require_finite=True,
            sim_require_nnan=True,
            nc=nc,
        )
        return tuple(outs)

    devices = jax.devices()[:NCORES]
    mesh = Mesh(np.asarray(devices), ("core",))
    in_specs = (PartitionSpec("core"),) * (n_params + len(out_names))
    out_specs = (PartitionSpec("core"),) * len(out_names)
    jitted = jax.jit(
        shard_map(
            _body, mesh=mesh, in_specs=in_specs, out_specs=out_specs, check_rep=False
        ),
        keep_unused=True,
    )
    sharding = NamedSharding(mesh, PartitionSpec("core"))
    zero_shapes = [
        ((NCORES * a.shape[0],) + tuple(a.shape[1:]), a.dtype) for a in out_avals
    ]

    def run(in_maps):
        concat_in = [
            np.concatenate([np.asarray(in_maps[c][nm]) for c in range(NCORES)], axis=0)
            for nm in in_names
        ]
        args = [jax.device_put(a, sharding) for a in concat_in] + [
            jax.device_put(np.zeros(shp, dt), sharding) for shp, dt in zero_shapes
        ]
        outs = jitted(*args)
        outs = [np.asarray(o) for o in outs]
        per_core = [
            {
                nm: outs[i].reshape(NCORES, *out_avals[i].shape)[c]
                for i, nm in enumerate(out_names)
            }
            for c in range(NCORES)
        ]
        return per_core

    _cached["runner"] = run
    _cached["jitted"] = jitted
    _cached["meta"] = (in_names, out_names, out_avals, sharding)
    return run


def kernel(x, Wqkv, bqkv, Wo, bo):
    run = _get_runner()
    in_maps = _host_shards(x, Wqkv, bqkv, Wo, bo)
    results = run(in_maps)

    out = np.empty((B, S, E), np.float32)
    for b in range(B):
        yT = np.concatenate(
            [results[2 * b]["out"], results[2 * b + 1]["out"]], axis=0
        )
        out[b] = yT.T
    return out



# revision 3
# speedup vs baseline: 1.1830x; 1.1830x over previous
"""Multi-head attention (B=4, S=2048, E=1024, H=16, D=64) on 8 TRN2 NeuronCores.

Sharding: data-parallel over batch (4) x tensor-parallel over heads (2).
Core c handles batch c//2 with heads [tp*8, tp*8+8), tp = c%2.

Per-core dataflow (all matmuls bf16 inputs, fp32 PSUM accumulation):
  phase 1: QKV projection.
    Q^T,K^T computed column-major ([head-dim, seq]) via lhsT=W, rhs=x^T.
    V computed row-major ([seq, head-dim]) via lhsT=x^T, rhs=Wv; a host-
    appended bias row on Wv plus an on-chip ones row implements +bias; an
    on-chip ones *column* appended to V makes the attention PV matmul also
    produce softmax row-sums.
  phase 2: per head: S^T = K^T-tiles @ Q^T (scores transposed, k on
    partitions), exp on ScalarE straight from PSUM (fused 1/8 scale, bf16
    out), O^T[65,2048] accumulated over 16 k-blocks where row 64 = softmax
    denominator l.  Normalize: partition-broadcast DMA of l, DVE reciprocal
    + multiply (fused bf16 cast).
  phase 3: out-proj partial y^T = Wo_shard^T-tiles @ O_n^T (+bias on tp0),
    ReduceScatter(add) over the TP pair, each rank keeps 512 rows of y^T.

All per-core bf16 operands (x^T, Wqk, Wv+bias row, Wo) are packed into ONE
flat DRAM input tensor and the two fp32 bias vectors into a second [128,16]
tensor: per-executable-arg dispatch overhead through the PJRT relay is
~90us/arg, so the arg count is kept minimal (data, bias, out).

Host: pre-transposes x, pre-slices/casts weights to bf16, packs, and
transposes the gathered y^T shards back to [B,S,E] fp32.
"""

import numpy as np
import ml_dtypes

B, S, E, H, D = 4, 2048, 1024, 16, 64
NCORES = 8
TP = 2
HPC = H // TP          # heads per core = 8
PAIRS = HPC // 2       # head pairs per core = 4
P = 128
KT = E // P            # 8 contraction tiles over E
SQ = S // 512          # 4 sequence chunks of 512
SB = S // P            # 16 sequence blocks of 128
CS = HPC * D           # per-core qkv col shard width = 512

_BF16 = ml_dtypes.bfloat16

# flat bf16 pack offsets (elements)
OFF_XT = 0                                 # [E, S]      = [1024, 2048]
OFF_WQK = OFF_XT + E * S                   # [E, 2*CS]   = [1024, 1024]
OFF_WV = OFF_WQK + E * 2 * CS              # [E+1, CS]   = [1025, 512]
OFF_WO = OFF_WV + (E + 1) * CS             # [CS, E]     = [512, 1024]
NDATA = OFF_WO + CS * E

_cached = {}


def _split_drain_waits(nc, mybir, max_waits=1):
    """This walrus build rejects instructions carrying more than ~2 sem
    waits; hoist extras onto preceding same-engine nops."""
    for f in nc.m.functions:
        for bb in f.blocks:
            insts = bb.instructions
            i = 0
            while i < len(insts):
                inst = insts[i]
                si = inst.sync_info
                if si is not None and len(si.on_wait) > max_waits:
                    extra = list(si.on_wait[max_waits:])
                    keep = list(si.on_wait[:max_waits])
                    for j, w in enumerate(extra):
                        nop = mybir.InstNoOp(
                            name=f"{inst.name}-waitsplit{j}", ins=[], outs=[]
                        )
                        nop.engine = inst.engine
                        nop.sync_info = mybir.SyncInfo(on_wait=[w], on_update=[])
                        nc.register_instruction(nop)
                        insts.insert(i, nop)
                        i += 1
                    inst.sync_info = mybir.SyncInfo(
                        on_wait=keep, on_update=list(si.on_update)
                    )
                i += 1


def _build_program(collective=True):
    import concourse.bass as bass
    import concourse.tile as tile
    from concourse import mybir

    f32 = mybir.dt.float32
    bf16 = mybir.dt.bfloat16

    nc = bass.Bass("TRN2", num_devices=NCORES, debug=False)

    data_d = nc.dram_tensor("data", [NDATA], bf16, kind="ExternalInput")
    bias_d = nc.dram_tensor("bias", [P, 16], f32, kind="ExternalInput")
    out_d = nc.dram_tensor("out", [E // TP, S], f32, kind="ExternalOutput")

    def dview(off, rows, cols):
        """[rows, cols] row-major view into the flat bf16 pack."""
        base = data_d.ap()
        return bass.AP(tensor=base.tensor, offset=base.offset + off,
                       ap=[[cols, rows], [1, cols]])

    groups = [[2 * i, 2 * i + 1] for i in range(NCORES // 2)]

    with tile.TileContext(nc) as tc:
        with (
            tc.tile_pool(name="const", bufs=1) as const,
            tc.tile_pool(name="win", bufs=1) as win,
            tc.tile_pool(name="qk", bufs=1) as qkp,
            tc.tile_pool(name="vsb", bufs=1) as vp,
            tc.tile_pool(name="on", bufs=1) as onp,
            tc.tile_pool(name="pt", bufs=6) as ptp,
            tc.tile_pool(name="rec", bufs=2) as recp,
            tc.tile_pool(name="ysb", bufs=4) as yp,
            tc.tile_pool(name="dram", bufs=1, space="DRAM") as dram,
        ):
            # ---- constants / weights into SBUF ----
            bias_sb = const.tile([P, 16], f32, tag="bias")
            nc.sync.dma_start(out=bias_sb[:], in_=bias_d.ap())
            bqk_sb = bias_sb[:, 0:8]
            bo_sb = bias_sb[:, 8:16]
            ones_sb = const.tile([1, S], bf16, tag="ones")
            nc.vector.memset(ones_sb[:], 1.0)

            wv_sb = [win.tile([P, CS], bf16, tag=f"wv{k}", name=f"wv{k}") for k in range(KT)]
            for k in range(KT):
                nc.sync.dma_start(out=wv_sb[k][:], in_=dview(OFF_WV + k * P * CS, P, CS))
            wvb_sb = win.tile([1, CS], bf16, tag="wvb")
            nc.sync.dma_start(out=wvb_sb[:], in_=dview(OFF_WV + E * CS, 1, CS))

            xt_sb = [win.tile([P, S], bf16, tag=f"xt{k}", name=f"xt{k}") for k in range(KT)]
            for k in range(KT):
                nc.sync.dma_start(out=xt_sb[k][:], in_=dview(OFF_XT + k * P * S, P, S))

            wqk_sb = [win.tile([P, 2 * CS], bf16, tag=f"wqk{k}", name=f"wqk{k}") for k in range(KT)]
            for k in range(KT):
                nc.sync.dma_start(
                    out=wqk_sb[k][:], in_=dview(OFF_WQK + k * P * 2 * CS, P, 2 * CS)
                )

            wo_sb = [win.tile([P, E], bf16, tag=f"wo{p}", name=f"wo{p}") for p in range(PAIRS)]
            for p in range(PAIRS):
                nc.sync.dma_start(out=wo_sb[p][:], in_=dview(OFF_WO + p * P * E, P, E))

            # persistent activations
            qk_sb = [qkp.tile([P, S], bf16, tag=f"qk{c}", name=f"qk{c}") for c in range(2 * CS // P)]
            v_sb = [vp.tile([P, HPC, D + 1], bf16, tag=f"v{s}", name=f"v{s}") for s in range(SB)]
            on_sb = [onp.tile([P, S], bf16, tag=f"on{p}", name=f"on{p}") for p in range(PAIRS)]

            # ---- shared PSUM pools (8 banks static across all phases) ----
            work_cm = tc.tile_pool(name="work", bufs=2, space="PSUM")
            work = work_cm.__enter__()
            acc_cm = tc.tile_pool(name="acc", bufs=2, space="PSUM")
            acc = acc_cm.__enter__()

            def emit_v_phase():
                # V: out[seq-block, vcols] ; lhsT = x^T tile, rhs = Wv tile
                for s in range(SB):
                    pv = work.tile([P, CS], f32, tag="w", name=f"pv{s}")
                    for k in range(KT):
                        nc.tensor.matmul(
                            pv[:],
                            xt_sb[k][:, s * P:(s + 1) * P],
                            wv_sb[k][:],
                            start=(k == 0),
                            stop=False,
                        )
                    # bias row: ones row (K=1) x Wv bias row
                    nc.tensor.matmul(
                        pv[:],
                        ones_sb[:, s * P:(s + 1) * P],
                        wvb_sb[:],
                        start=False,
                        stop=True,
                    )
                    nc.vector.memset(v_sb[s][:, :, D:D + 1], 1.0)
                    nc.vector.tensor_copy(v_sb[s][:, :, 0:D], pv[:])

            def qk_chain(c, q):
                def chain():
                    pq = work.tile([P, 512], f32, tag="w", name=f"pq{c}_{q}")
                    for k in range(KT):
                        nc.tensor.matmul(
                            pq[:],
                            wqk_sb[k][:, c * P:(c + 1) * P],
                            xt_sb[k][:, q * 512:(q + 1) * 512],
                            start=(k == 0),
                            stop=(k == KT - 1),
                        )
                    nc.vector.tensor_scalar_add(
                        qk_sb[c][:, q * 512:(q + 1) * 512],
                        pq[:],
                        bqk_sb[:, c:c + 1],
                    )
                return chain

            def emit_qk_pair(p):
                for c in (p, PAIRS + p):
                    for q in range(SQ):
                        qk_chain(c, q)()

            def emit_head(h):
                p, half = h // 2, h % 2
                r0 = half * D
                qT = qk_sb[p]
                kT = qk_sb[PAIRS + p]
                po = [
                    acc.tile([D + 1, 1024], f32, tag="a", name=f"po{h}_{j}")
                    for j in range(2)
                ]
                for kb in range(SB):
                    pt = ptp.tile([P, S], bf16, tag="pt", name=f"pt{h}_{kb}")
                    for qh in range(2):
                        ps = work.tile([P, 1024], f32, tag="w", name=f"ps{h}_{kb}_{qh}")
                        for q2 in range(2):
                            q = 2 * qh + q2
                            nc.tensor.matmul(
                                ps[:, q2 * 512:(q2 + 1) * 512],
                                kT[r0:r0 + D, kb * P:(kb + 1) * P],
                                qT[r0:r0 + D, q * 512:(q + 1) * 512],
                                start=True,
                                stop=True,
                            )
                        nc.scalar.activation(
                            pt[:, qh * 1024:(qh + 1) * 1024],
                            ps[:],
                            mybir.ActivationFunctionType.Exp,
                            scale=0.125,
                        )
                        for q2 in range(2):
                            o = qh * 1024 + q2 * 512
                            nc.tensor.matmul(
                                po[qh][:, q2 * 512:(q2 + 1) * 512],
                                v_sb[kb][:, h, :],
                                pt[:, o:o + 512],
                                start=(kb == 0),
                                stop=(kb == SB - 1),
                            )
                # normalize: O^T[0:D] / l (l = row D), write bf16
                for qh in range(2):
                    lsb = recp.tile([1, 1024], f32, tag="lsb", name=f"lsb{h}_{qh}")
                    nc.vector.reciprocal(lsb[:], po[qh][D:D + 1, :])
                    lscr = dram.tile(
                        [1, 1024], f32, tag="lscr", name=f"lscr{h}_{qh}", bufs=2
                    )
                    nc.sync.dma_start(out=lscr[:], in_=lsb[:])
                    ldr = lscr[:]
                    lbc = bass.AP(
                        tensor=ldr.tensor,
                        offset=ldr.offset,
                        ap=[[0, D]] + [list(x) for x in ldr.ap[1:]],
                    )
                    rin = recp.tile([D, 1024], f32, tag="rin", name=f"rin{h}_{qh}")
                    nc.sync.dma_start(out=rin[:], in_=lbc)
                    nc.vector.tensor_mul(
                        on_sb[p][r0:r0 + D, qh * 1024:(qh + 1) * 1024],
                        po[qh][0:D, :],
                        rin[:],
                    )

            # interleave: QK pair 0 first so head 0's S-matmuls (and exp)
            # start ASAP; V is emitted after head 0 so its chains fill PE
            # whenever head 0 blocks (head 0's O-phase pulls v_sb[kb] just in
            # time).  Later QK pairs ride between heads.
            emit_v_phase()
            emit_qk_pair(0)
            emit_head(0)
            emit_head(1)
            emit_qk_pair(1)
            emit_head(2)
            emit_head(3)
            emit_qk_pair(2)
            emit_head(4)
            emit_head(5)
            emit_qk_pair(3)
            emit_head(6)
            emit_head(7)

            # ---- phase 3: output projection + reduce-scatter ----
            # chunk-major layout: each sequence chunk is a contiguous block
            # (collective inputs must be contiguous)
            y_dram = dram.tile([SQ, E, 512], f32, tag="ydram")
            y_red = dram.tile([SQ, E // TP, 512], f32, tag="yred")
            for q in range(SQ):
                for e in range(E // P):
                    py = work.tile([P, 512], f32, tag="w", name=f"py{e}_{q}")
                    for p in range(PAIRS):
                        nc.tensor.matmul(
                            py[:],
                            wo_sb[p][:, e * P:(e + 1) * P],
                            on_sb[p][:, q * 512:(q + 1) * 512],
                            start=(p == 0),
                            stop=(p == PAIRS - 1),
                        )
                    ye = yp.tile([P, 512], f32, tag="ysb")
                    nc.vector.tensor_scalar_add(ye[:], py[:], bo_sb[:, e:e + 1])
                    nc.sync.dma_start(
                        out=y_dram[q, e * P:(e + 1) * P, :],
                        in_=ye[:],
                    )
                # reduce-scatter this sequence chunk while the next computes
                if collective:
                    nc.gpsimd.collective_compute(
                        "ReduceScatter",
                        mybir.AluOpType.add,
                        replica_groups=groups,
                        ins=[y_dram[q].opt()],
                        outs=[y_red[q].opt()],
                    )
                    nc.sync.dma_start(
                        out=out_d[:, q * 512:(q + 1) * 512],
                        in_=y_red[q],
                    )
                else:
                    nc.sync.dma_start(
                        out=out_d[:, q * 512:(q + 1) * 512],
                        in_=y_dram[q, 0:E // TP, :],
                    )

            acc_cm.__exit__(None, None, None)
            work_cm.__exit__(None, None, None)

    from concourse import mybir as _mybir
    _split_drain_waits(nc, _mybir)
    return nc


def _host_shards(x, Wqkv, bqkv, Wo, bo):
    x = np.asarray(x, np.float32)
    Wqkv = np.asarray(Wqkv, np.float32)
    bqkv = np.asarray(bqkv, np.float32)
    Wo = np.asarray(Wo, np.float32)
    bo = np.asarray(bo, np.float32)

    in_maps = []
    for c in range(NCORES):
        b, tp = c // 2, c % 2
        lo = tp * CS
        data = np.empty(NDATA, _BF16)
        data[OFF_XT:OFF_XT + E * S] = x[b].T.astype(_BF16).ravel()
        data[OFF_WQK:OFF_WQK + E * 2 * CS] = (
            np.concatenate([Wqkv[:, lo:lo + CS], Wqkv[:, E + lo:E + lo + CS]], axis=1)
            .astype(_BF16)
            .ravel()
        )
        data[OFF_WV:OFF_WV + (E + 1) * CS] = (
            np.concatenate(
                [
                    Wqkv[:, 2 * E + lo:2 * E + lo + CS],
                    bqkv[None, 2 * E + lo:2 * E + lo + CS],
                ],
                axis=0,
            )
            .astype(_BF16)
            .ravel()
        )
        data[OFF_WO:OFF_WO + CS * E] = Wo[lo:lo + CS, :].astype(_BF16).ravel()

        bqk = (
            np.concatenate([bqkv[lo:lo + CS], bqkv[E + lo:E + lo + CS]])
            .reshape(2 * CS // P, P)
            .T.astype(np.float32)
        )
        bo_c = bo if tp == 0 else np.zeros_like(bo)
        bo_c = bo_c.reshape(E // P, P).T.astype(np.float32)
        bias = np.concatenate([bqk, bo_c], axis=1)
        in_maps.append(
            {"data": data, "bias": np.ascontiguousarray(bias)}
        )
    return in_maps


def _get_runner():
    """Build the Bass program once and wrap it in a cached 8-core jitted
    callable (same execution path run_bass_kernel_spmd uses under axon, but
    the XLA executable is reused across kernel() calls)."""
    if "runner" in _cached:
        return _cached["runner"]

    import jax
    from jax.sharding import Mesh, PartitionSpec, NamedSharding
    from jax.experimental.shard_map import shard_map
    from concourse import bass2jax, mybir

    nc = _build_program()
    _cached["nc"] = nc
    bass2jax.install_neuronx_cc_hook()

    partition_name = nc.partition_id_tensor.name if nc.partition_id_tensor else None
    in_names, out_names, out_avals = [], [], []
    for alloc in nc.m.functions[0].allocations:
        if not isinstance(alloc, mybir.MemoryLocationSet):
            continue
        name = alloc.memorylocations[0].name
        if alloc.kind == "ExternalInput":
            if name != partition_name:
                in_names.append(name)
        elif alloc.kind == "ExternalOutput":
            out_names.append(name)
            out_avals.append(
                jax.core.ShapedArray(tuple(alloc.tensor_shape), mybir.dt.np(alloc.dtype))
            )
    n_params = len(in_names)
    all_in_names = list(in_names) + list(out_names)
    if partition_name is not None:
        all_in_names.append(partition_name)

    def _body(*args):
        operands = list(args)
        if partition_name is not None:
            operands.append(bass2jax.partition_id_tensor())
        outs = bass2jax._bass_exec_p.bind(
            *operands,
            out_avals=tuple(out_avals),
            in_names=tuple(all_in_names),
            out_names=tuple(out_names),
            lowering_input_output_aliases=(),
            sim_require_finite=True,
            sim_require_nnan=True,
            nc=nc,
        )
        return tuple(outs)

    devices = jax.devices()[:NCORES]
    mesh = Mesh(np.asarray(devices), ("core",))
    in_specs = (PartitionSpec("core"),) * (n_params + len(out_names))
    out_specs = (PartitionSpec("core"),) * len(out_names)
    jitted = jax.jit(
        shard_map(
            _body, mesh=mesh, in_specs=in_specs, out_specs=out_specs, check_rep=False
        ),
        keep_unused=True,
    )
    sharding = NamedSharding(mesh, PartitionSpec("core"))
    zero_shapes = [
        ((NCORES * a.shape[0],) + tuple(a.shape[1:]), a.dtype) for a in out_avals
    ]

    def run(in_maps):
        concat_in = [
            np.concatenate([np.asarray(in_maps[c][nm]) for c in range(NCORES)], axis=0)
            for nm in in_names
        ]
        args = [jax.device_put(a, sharding) for a in concat_in] + [
            jax.device_put(np.zeros(shp, dt), sharding) for shp, dt in zero_shapes
        ]
        outs = jitted(*args)
        outs = [np.asarray(o) for o in outs]
        per_core = [
            {
                nm: outs[i].reshape(NCORES, *out_avals[i].shape)[c]
                for i, nm in enumerate(out_names)
            }
            for c in range(NCORES)
        ]
        return per_core

    _cached["runner"] = run
    _cached["jitted"] = jitted
    _cached["meta"] = (in_names, out_names, out_avals, sharding)
    return run


def kernel(x, Wqkv, bqkv, Wo, bo):
    run = _get_runner()
    in_maps = _host_shards(x, Wqkv, bqkv, Wo, bo)
    results = run(in_maps)

    out = np.empty((B, S, E), np.float32)
    for b in range(B):
        yT = np.concatenate(
            [results[2 * b]["out"], results[2 * b + 1]["out"]], axis=0
        )
        out[b] = yT.T
    return out


# revision 8
# speedup vs baseline: 2.7239x; 2.3025x over previous
"""Multi-head attention (B=4, S=2048, E=1024, H=16, D=64) on 8 TRN2 NeuronCores.

Sharding: data-parallel over batch (4) x tensor-parallel over heads (2).
Core c handles batch c//2 with heads [tp*8, tp*8+8), tp = c%2.

Per-core dataflow (all matmuls bf16 inputs, fp32 PSUM accumulation):
  phase 1: QKV projection.
    Q^T,K^T computed column-major ([head-dim, seq]) via lhsT=W, rhs=x^T.
    V computed row-major ([seq, head-dim]) via lhsT=x^T, rhs=Wv; a host-
    appended bias row on Wv plus an on-chip ones row implements +bias; an
    on-chip ones *column* appended to V makes the attention PV matmul also
    produce softmax row-sums.
  phase 2: per head: S^T = K^T-tiles @ Q^T (scores transposed, k on
    partitions), exp on ScalarE straight from PSUM (fused 1/8 scale, bf16
    out), O^T[65,2048] accumulated over 16 k-blocks where row 64 = softmax
    denominator l.  Normalize: partition-broadcast DMA of l, DVE reciprocal
    + multiply (fused bf16 cast).
  phase 3: out-proj partial y^T = Wo_shard^T-tiles @ O_n^T (+bias on tp0),
    ReduceScatter(add) over the TP pair, each rank keeps 512 rows of y^T.

All per-core bf16 operands (x^T, Wqk, Wv+bias row, Wo) are packed into ONE
flat DRAM input tensor and the two fp32 bias vectors into a second [128,16]
tensor: per-executable-arg dispatch overhead through the PJRT relay is
~90us/arg, so the arg count is kept minimal (data, bias, out).

Host: pre-transposes x, pre-slices/casts weights to bf16, packs, and
transposes the gathered y^T shards back to [B,S,E] fp32.
"""

import numpy as np
import ml_dtypes

B, S, E, H, D = 4, 2048, 1024, 16, 64
NCORES = 8
TP = 2
HPC = H // TP          # heads per core = 8
PAIRS = HPC // 2       # head pairs per core = 4
P = 128
KT = E // P            # 8 contraction tiles over E
SQ = S // 512          # 4 sequence chunks of 512
SB = S // P            # 16 sequence blocks of 128
CS = HPC * D           # per-core qkv col shard width = 512

_BF16 = ml_dtypes.bfloat16

# flat bf16 pack offsets (elements)
OFF_XT = 0                                 # [E, S]      = [1024, 2048]
OFF_WQK = OFF_XT + E * S                   # [E, 2*CS]   = [1024, 1024]
OFF_WV = OFF_WQK + E * 2 * CS              # [E+1, CS]   = [1025, 512]
OFF_WO = OFF_WV + (E + 1) * CS             # [CS, E]     = [512, 1024]
OFF_BIAS = OFF_WO + CS * E                 # [P, 32]: fp32 biases split into
NDATA = OFF_BIAS + P * 32                  # bf16 hi [P,16] ++ lo [P,16]

_cached = {}


def _split_drain_waits(nc, mybir, max_waits=1):
    """This walrus build rejects instructions carrying more than ~2 sem
    waits; hoist extras onto preceding same-engine nops."""
    for f in nc.m.functions:
        for bb in f.blocks:
            insts = bb.instructions
            i = 0
            while i < len(insts):
                inst = insts[i]
                si = inst.sync_info
                if si is not None and len(si.on_wait) > max_waits:
                    extra = list(si.on_wait[max_waits:])
                    keep = list(si.on_wait[:max_waits])
                    for j, w in enumerate(extra):
                        nop = mybir.InstNoOp(
                            name=f"{inst.name}-waitsplit{j}", ins=[], outs=[]
                        )
                        nop.engine = inst.engine
                        nop.sync_info = mybir.SyncInfo(on_wait=[w], on_update=[])
                        nc.register_instruction(nop)
                        insts.insert(i, nop)
                        i += 1
                    inst.sync_info = mybir.SyncInfo(
                        on_wait=keep, on_update=list(si.on_update)
                    )
                i += 1


def _build_program(collective=True):
    import concourse.bass as bass
    import concourse.tile as tile
    from concourse import mybir

    f32 = mybir.dt.float32
    bf16 = mybir.dt.bfloat16

    nc = bass.Bass("TRN2", num_devices=NCORES, debug=False)

    data_d = nc.dram_tensor("data", [NDATA], bf16, kind="ExternalInput")
    out_d = nc.dram_tensor("out", [E // TP, S], f32, kind="ExternalOutput")

    def dview(off, rows, cols):
        """[rows, cols] row-major view into the flat bf16 pack."""
        base = data_d.ap()
        return bass.AP(tensor=base.tensor, offset=base.offset + off,
                       ap=[[cols, rows], [1, cols]])

    groups = [[2 * i, 2 * i + 1] for i in range(NCORES // 2)]

    with tile.TileContext(nc) as tc:
        with (
            tc.tile_pool(name="const", bufs=1) as const,
            tc.tile_pool(name="win", bufs=1) as win,
            tc.tile_pool(name="qk", bufs=1) as qkp,
            tc.tile_pool(name="vsb", bufs=1) as vp,
            tc.tile_pool(name="on", bufs=1) as onp,
            tc.tile_pool(name="pt", bufs=6) as ptp,
            tc.tile_pool(name="rec", bufs=2) as recp,
            tc.tile_pool(name="ysb", bufs=4) as yp,
            tc.tile_pool(name="dram", bufs=1, space="DRAM") as dram,
        ):
            # ---- constants / weights into SBUF ----
            # fp32 biases travel as bf16 hi/lo halves inside the bf16 pack;
            # recombine exactly: f32 = hi + lo (lo = f32 - f32(hi))
            bias_hl = const.tile([P, 32], bf16, tag="bias_hl")
            nc.sync.dma_start(out=bias_hl[:], in_=dview(OFF_BIAS, P, 32))
            bias_sb = const.tile([P, 16], f32, tag="bias")
            nc.vector.tensor_add(bias_sb[:], bias_hl[:, 0:16], bias_hl[:, 16:32])
            bqk_sb = bias_sb[:, 0:8]
            bo_sb = bias_sb[:, 8:16]
            ones_sb = const.tile([1, S], bf16, tag="ones")
            nc.vector.memset(ones_sb[:], 1.0)

            wv_sb = [win.tile([P, CS], bf16, tag=f"wv{k}", name=f"wv{k}") for k in range(KT)]
            for k in range(KT):
                nc.sync.dma_start(out=wv_sb[k][:], in_=dview(OFF_WV + k * P * CS, P, CS))
            wvb_sb = win.tile([1, CS], bf16, tag="wvb")
            nc.sync.dma_start(out=wvb_sb[:], in_=dview(OFF_WV + E * CS, 1, CS))

            xt_sb = [win.tile([P, S], bf16, tag=f"xt{k}", name=f"xt{k}") for k in range(KT)]
            for k in range(KT):
                nc.sync.dma_start(out=xt_sb[k][:], in_=dview(OFF_XT + k * P * S, P, S))

            wqk_sb = [win.tile([P, 2 * CS], bf16, tag=f"wqk{k}", name=f"wqk{k}") for k in range(KT)]
            for k in range(KT):
                nc.sync.dma_start(
                    out=wqk_sb[k][:], in_=dview(OFF_WQK + k * P * 2 * CS, P, 2 * CS)
                )

            wo_sb = [win.tile([P, E], bf16, tag=f"wo{p}", name=f"wo{p}") for p in range(PAIRS)]
            for p in range(PAIRS):
                nc.sync.dma_start(out=wo_sb[p][:], in_=dview(OFF_WO + p * P * E, P, E))

            # persistent activations
            qk_sb = [qkp.tile([P, S], bf16, tag=f"qk{c}", name=f"qk{c}") for c in range(2 * CS // P)]
            v_sb = [vp.tile([P, HPC, D + 1], bf16, tag=f"v{s}", name=f"v{s}") for s in range(SB)]
            on_sb = [onp.tile([P, S], bf16, tag=f"on{p}", name=f"on{p}") for p in range(PAIRS)]

            # ---- shared PSUM pools (8 banks static across all phases) ----
            work_cm = tc.tile_pool(name="work", bufs=2, space="PSUM")
            work = work_cm.__enter__()
            acc_cm = tc.tile_pool(name="acc", bufs=2, space="PSUM")
            acc = acc_cm.__enter__()

            def emit_v_phase():
                # V: out[seq-block, vcols] ; lhsT = x^T tile, rhs = Wv tile
                for s in range(SB):
                    pv = work.tile([P, CS], f32, tag="w", name=f"pv{s}")
                    for k in range(KT):
                        nc.tensor.matmul(
                            pv[:],
                            xt_sb[k][:, s * P:(s + 1) * P],
                            wv_sb[k][:],
                            start=(k == 0),
                            stop=False,
                        )
                    # bias row: ones row (K=1) x Wv bias row
                    nc.tensor.matmul(
                        pv[:],
                        ones_sb[:, s * P:(s + 1) * P],
                        wvb_sb[:],
                        start=False,
                        stop=True,
                    )
                    nc.vector.memset(v_sb[s][:, :, D:D + 1], 1.0)
                    nc.vector.tensor_copy(v_sb[s][:, :, 0:D], pv[:])

            def qk_chain(c, q):
                def chain():
                    pq = work.tile([P, 512], f32, tag="w", name=f"pq{c}_{q}")
                    for k in range(KT):
                        nc.tensor.matmul(
                            pq[:],
                            wqk_sb[k][:, c * P:(c + 1) * P],
                            xt_sb[k][:, q * 512:(q + 1) * 512],
                            start=(k == 0),
                            stop=(k == KT - 1),
                        )
                    nc.vector.tensor_scalar_add(
                        qk_sb[c][:, q * 512:(q + 1) * 512],
                        pq[:],
                        bqk_sb[:, c:c + 1],
                    )
                return chain

            def emit_qk_pair(p):
                for c in (p, PAIRS + p):
                    for q in range(SQ):
                        qk_chain(c, q)()

            def emit_head(h):
                p, half = h // 2, h % 2
                r0 = half * D
                qT = qk_sb[p]
                kT = qk_sb[PAIRS + p]
                po = [
                    acc.tile([D + 1, 1024], f32, tag="a", name=f"po{h}_{j}")
                    for j in range(2)
                ]
                for kb in range(SB):
                    pt = ptp.tile([P, S], bf16, tag="pt", name=f"pt{h}_{kb}")
                    for qh in range(2):
                        ps = work.tile([P, 1024], f32, tag="w", name=f"ps{h}_{kb}_{qh}")
                        for q2 in range(2):
                            q = 2 * qh + q2
                            nc.tensor.matmul(
                                ps[:, q2 * 512:(q2 + 1) * 512],
                                kT[r0:r0 + D, kb * P:(kb + 1) * P],
                                qT[r0:r0 + D, q * 512:(q + 1) * 512],
                                start=True,
                                stop=True,
                            )
                        nc.scalar.activation(
                            pt[:, qh * 1024:(qh + 1) * 1024],
                            ps[:],
                            mybir.ActivationFunctionType.Exp,
                            scale=0.125,
                        )
                        for q2 in range(2):
                            o = qh * 1024 + q2 * 512
                            nc.tensor.matmul(
                                po[qh][:, q2 * 512:(q2 + 1) * 512],
                                v_sb[kb][:, h, :],
                                pt[:, o:o + 512],
                                start=(kb == 0),
                                stop=(kb == SB - 1),
                            )
                # normalize: O^T[0:D] / l (l = row D), write bf16
                for qh in range(2):
                    lsb = recp.tile([1, 1024], f32, tag="lsb", name=f"lsb{h}_{qh}")
                    nc.vector.reciprocal(lsb[:], po[qh][D:D + 1, :])
                    lscr = dram.tile(
                        [1, 1024], f32, tag="lscr", name=f"lscr{h}_{qh}", bufs=2
                    )
                    nc.sync.dma_start(out=lscr[:], in_=lsb[:])
                    ldr = lscr[:]
                    lbc = bass.AP(
                        tensor=ldr.tensor,
                        offset=ldr.offset,
                        ap=[[0, D]] + [list(x) for x in ldr.ap[1:]],
                    )
                    rin = recp.tile([D, 1024], f32, tag="rin", name=f"rin{h}_{qh}")
                    nc.sync.dma_start(out=rin[:], in_=lbc)
                    nc.vector.tensor_mul(
                        on_sb[p][r0:r0 + D, qh * 1024:(qh + 1) * 1024],
                        po[qh][0:D, :],
                        rin[:],
                    )

            # interleave: QK pair 0 first so head 0's S-matmuls (and exp)
            # start ASAP; V is emitted after head 0 so its chains fill PE
            # whenever head 0 blocks (head 0's O-phase pulls v_sb[kb] just in
            # time).  Later QK pairs ride between heads.
            emit_v_phase()
            emit_qk_pair(0)
            emit_head(0)
            emit_head(1)
            emit_qk_pair(1)
            emit_head(2)
            emit_head(3)
            emit_qk_pair(2)
            emit_head(4)
            emit_head(5)
            emit_qk_pair(3)
            emit_head(6)
            emit_head(7)

            # ---- phase 3: output projection + reduce-scatter ----
            # chunk-major layout: each sequence chunk is a contiguous block
            # (collective inputs must be contiguous)
            y_dram = dram.tile([SQ, E, 512], f32, tag="ydram")
            y_red = dram.tile([SQ, E // TP, 512], f32, tag="yred")
            for q in range(SQ):
                for e in range(E // P):
                    py = work.tile([P, 512], f32, tag="w", name=f"py{e}_{q}")
                    for p in range(PAIRS):
                        nc.tensor.matmul(
                            py[:],
                            wo_sb[p][:, e * P:(e + 1) * P],
                            on_sb[p][:, q * 512:(q + 1) * 512],
                            start=(p == 0),
                            stop=(p == PAIRS - 1),
                        )
                    ye = yp.tile([P, 512], f32, tag="ysb")
                    nc.vector.tensor_scalar_add(ye[:], py[:], bo_sb[:, e:e + 1])
                    nc.sync.dma_start(
                        out=y_dram[q, e * P:(e + 1) * P, :],
                        in_=ye[:],
                    )
                # reduce-scatter this sequence chunk while the next computes
                if collective:
                    nc.gpsimd.collective_compute(
                        "ReduceScatter",
                        mybir.AluOpType.add,
                        replica_groups=groups,
                        ins=[y_dram[q].opt()],
                        outs=[y_red[q].opt()],
                    )
                    nc.sync.dma_start(
                        out=out_d[:, q * 512:(q + 1) * 512],
                        in_=y_red[q],
                    )
                else:
                    nc.sync.dma_start(
                        out=out_d[:, q * 512:(q + 1) * 512],
                        in_=y_dram[q, 0:E // TP, :],
                    )

            acc_cm.__exit__(None, None, None)
            work_cm.__exit__(None, None, None)

    from concourse import mybir as _mybir
    _split_drain_waits(nc, _mybir)
    return nc


def _host_shards(x, Wqkv, bqkv, Wo, bo):
    x = np.asarray(x, np.float32)
    Wqkv = np.asarray(Wqkv, np.float32)
    bqkv = np.asarray(bqkv, np.float32)
    Wo = np.asarray(Wo, np.float32)
    bo = np.asarray(bo, np.float32)

    in_maps = []
    for c in range(NCORES):
        b, tp = c // 2, c % 2
        lo = tp * CS
        data = np.empty(NDATA, _BF16)
        data[OFF_XT:OFF_XT + E * S] = x[b].T.astype(_BF16).ravel()
        data[OFF_WQK:OFF_WQK + E * 2 * CS] = (
            np.concatenate([Wqkv[:, lo:lo + CS], Wqkv[:, E + lo:E + lo + CS]], axis=1)
            .astype(_BF16)
            .ravel()
        )
        data[OFF_WV:OFF_WV + (E + 1) * CS] = (
            np.concatenate(
                [
                    Wqkv[:, 2 * E + lo:2 * E + lo + CS],
                    bqkv[None, 2 * E + lo:2 * E + lo + CS],
                ],
                axis=0,
            )
            .astype(_BF16)
            .ravel()
        )
        data[OFF_WO:OFF_WO + CS * E] = Wo[lo:lo + CS, :].astype(_BF16).ravel()

        bqk = (
            np.concatenate([bqkv[lo:lo + CS], bqkv[E + lo:E + lo + CS]])
            .reshape(2 * CS // P, P)
            .T.astype(np.float32)
        )
        bo_c = bo if tp == 0 else np.zeros_like(bo)
        bo_c = bo_c.reshape(E // P, P).T.astype(np.float32)
        bias = np.concatenate([bqk, bo_c], axis=1)
        hi = bias.astype(_BF16)
        lo_ = (bias - hi.astype(np.float32)).astype(_BF16)
        data[OFF_BIAS:NDATA] = np.concatenate([hi, lo_], axis=1).ravel()
        in_maps.append({"data": data})
    return in_maps


def _get_runner():
    """Build the Bass program once and wrap it in a cached 8-core jitted
    callable (same execution path run_bass_kernel_spmd uses under axon, but
    the XLA executable is reused across kernel() calls)."""
    if "runner" in _cached:
        return _cached["runner"]

    import jax
    from jax.sharding import Mesh, PartitionSpec, NamedSharding
    from jax.experimental.shard_map import shard_map
    from concourse import bass2jax, mybir

    nc = _build_program()
    _cached["nc"] = nc
    bass2jax.install_neuronx_cc_hook()

    partition_name = nc.partition_id_tensor.name if nc.partition_id_tensor else None
    in_names, out_names, out_avals = [], [], []
    for alloc in nc.m.functions[0].allocations:
        if not isinstance(alloc, mybir.MemoryLocationSet):
            continue
        name = alloc.memorylocations[0].name
        if alloc.kind == "ExternalInput":
            if name != partition_name:
                in_names.append(name)
        elif alloc.kind == "ExternalOutput":
            out_names.append(name)
            out_avals.append(
                jax.core.ShapedArray(tuple(alloc.tensor_shape), mybir.dt.np(alloc.dtype))
            )
    n_params = len(in_names)
    all_in_names = list(in_names) + list(out_names)
    if partition_name is not None:
        all_in_names.append(partition_name)

    def _body(*args):
        operands = list(args)
        if partition_name is not None:
            operands.append(bass2jax.partition_id_tensor())
        outs = bass2jax._bass_exec_p.bind(
            *operands,
            out_avals=tuple(out_avals),
            in_names=tuple(all_in_names),
            out_names=tuple(out_names),
            lowering_input_output_aliases=(),
            sim_require_finite=True,
            sim_require_nnan=True,
            nc=nc,
        )
        return tuple(outs)

    devices = jax.devices()[:NCORES]
    mesh = Mesh(np.asarray(devices), ("core",))
    in_specs = (PartitionSpec("core"),) * (n_params + len(out_names))
    out_specs = (PartitionSpec("core"),) * len(out_names)
    jitted = jax.jit(
        shard_map(
            _body, mesh=mesh, in_specs=in_specs, out_specs=out_specs, check_rep=False
        ),
        keep_unused=True,
    )
    sharding = NamedSharding(mesh, PartitionSpec("core"))
    zero_shapes = [
        ((NCORES * a.shape[0],) + tuple(a.shape[1:]), a.dtype) for a in out_avals
    ]

    def run(in_maps):
        concat_in = [
            np.concatenate([np.asarray(in_maps[c][nm]) for c in range(NCORES)], axis=0)
            for nm in in_names
        ]
        args = [jax.device_put(a, sharding) for a in concat_in] + [
            jax.device_put(np.zeros(shp, dt), sharding) for shp, dt in zero_shapes
        ]
        outs = jitted(*args)
        outs = [np.asarray(o) for o in outs]
        per_core = [
            {
                nm: outs[i].reshape(NCORES, *out_avals[i].shape)[c]
                for i, nm in enumerate(out_names)
            }
            for c in range(NCORES)
        ]
        return per_core

    _cached["runner"] = run
    _cached["jitted"] = jitted
    _cached["meta"] = (in_names, out_names, out_avals, sharding)
    return run


def kernel(x, Wqkv, bqkv, Wo, bo):
    run = _get_runner()
    in_maps = _host_shards(x, Wqkv, bqkv, Wo, bo)
    results = run(in_maps)

    out = np.empty((B, S, E), np.float32)
    for b in range(B):
        yT = np.concatenate(
            [results[2 * b]["out"], results[2 * b + 1]["out"]], axis=0
        )
        out[b] = yT.T
    return out


# revision 9
# speedup vs baseline: 2.7620x; 1.0140x over previous
"""Multi-head attention (B=4, S=2048, E=1024, H=16, D=64) on 8 TRN2 NeuronCores.

Sharding: data-parallel over batch (4) x tensor-parallel over heads (2).
Core c handles batch c//2 with heads [tp*8, tp*8+8), tp = c%2.

Per-core dataflow (all matmuls bf16 inputs, fp32 PSUM accumulation):
  phase 1: QKV projection.
    Q^T,K^T computed column-major ([head-dim, seq]) via lhsT=W, rhs=x^T.
    V computed row-major ([seq, head-dim]) via lhsT=x^T, rhs=Wv; a host-
    appended bias row on Wv plus an on-chip ones row implements +bias; an
    on-chip ones *column* appended to V makes the attention PV matmul also
    produce softmax row-sums.
  phase 2: per head: S^T = K^T-tiles @ Q^T (scores transposed, k on
    partitions), exp on ScalarE straight from PSUM (fused 1/8 scale, bf16
    out), O^T[65,2048] accumulated over 16 k-blocks where row 64 = softmax
    denominator l.  Normalize: partition-broadcast DMA of l, DVE reciprocal
    + multiply (fused bf16 cast).
  phase 3: out-proj partial y^T = Wo_shard^T-tiles @ O_n^T (+bias on tp0),
    ReduceScatter(add) over the TP pair, each rank keeps 512 rows of y^T.

All per-core bf16 operands (x^T, Wqk, Wv+bias row, Wo) are packed into ONE
flat DRAM input tensor and the two fp32 bias vectors into a second [128,16]
tensor: per-executable-arg dispatch overhead through the PJRT relay is
~90us/arg, so the arg count is kept minimal (data, bias, out).

Host: pre-transposes x, pre-slices/casts weights to bf16, packs, and
transposes the gathered y^T shards back to [B,S,E] fp32.
"""

import numpy as np
import ml_dtypes

B, S, E, H, D = 4, 2048, 1024, 16, 64
NCORES = 8
TP = 2
HPC = H // TP          # heads per core = 8
PAIRS = HPC // 2       # head pairs per core = 4
P = 128
KT = E // P            # 8 contraction tiles over E
SQ = S // 512          # 4 sequence chunks of 512
SB = S // P            # 16 sequence blocks of 128
CS = HPC * D           # per-core qkv col shard width = 512

_BF16 = ml_dtypes.bfloat16

# flat bf16 pack offsets (elements)
OFF_XT = 0                                 # [E, S]      = [1024, 2048]
OFF_WQK = OFF_XT + E * S                   # [E, 2*CS]   = [1024, 1024]
OFF_WV = OFF_WQK + E * 2 * CS              # [E+1, CS]   = [1025, 512]
OFF_WO = OFF_WV + (E + 1) * CS             # [CS, E]     = [512, 1024]
OFF_BIAS = OFF_WO + CS * E                 # [P, 32]: fp32 biases split into
NDATA = OFF_BIAS + P * 32                  # bf16 hi [P,16] ++ lo [P,16]

_cached = {}


def _split_drain_waits(nc, mybir, max_waits=1):
    """This walrus build rejects instructions carrying more than ~2 sem
    waits; hoist extras onto preceding same-engine nops."""
    for f in nc.m.functions:
        for bb in f.blocks:
            insts = bb.instructions
            i = 0
            while i < len(insts):
                inst = insts[i]
                si = inst.sync_info
                if si is not None and len(si.on_wait) > max_waits:
                    extra = list(si.on_wait[max_waits:])
                    keep = list(si.on_wait[:max_waits])
                    for j, w in enumerate(extra):
                        nop = mybir.InstNoOp(
                            name=f"{inst.name}-waitsplit{j}", ins=[], outs=[]
                        )
                        nop.engine = inst.engine
                        nop.sync_info = mybir.SyncInfo(on_wait=[w], on_update=[])
                        nc.register_instruction(nop)
                        insts.insert(i, nop)
                        i += 1
                    inst.sync_info = mybir.SyncInfo(
                        on_wait=keep, on_update=list(si.on_update)
                    )
                i += 1


def _build_program(collective=True):
    import concourse.bass as bass
    import concourse.tile as tile
    from concourse import mybir

    f32 = mybir.dt.float32
    bf16 = mybir.dt.bfloat16

    nc = bass.Bass("TRN2", num_devices=NCORES, debug=False)

    data_d = nc.dram_tensor("data", [NDATA], bf16, kind="ExternalInput")
    out_d = nc.dram_tensor("out", [E // TP, S], f32, kind="ExternalOutput")

    def dview(off, rows, cols):
        """[rows, cols] row-major view into the flat bf16 pack."""
        base = data_d.ap()
        return bass.AP(tensor=base.tensor, offset=base.offset + off,
                       ap=[[cols, rows], [1, cols]])

    groups = [[2 * i, 2 * i + 1] for i in range(NCORES // 2)]

    with tile.TileContext(nc) as tc:
        with (
            tc.tile_pool(name="const", bufs=1) as const,
            tc.tile_pool(name="win", bufs=1) as win,
            tc.tile_pool(name="qk", bufs=1) as qkp,
            tc.tile_pool(name="vsb", bufs=1) as vp,
            tc.tile_pool(name="on", bufs=1) as onp,
            tc.tile_pool(name="pt", bufs=6) as ptp,
            tc.tile_pool(name="rec", bufs=2) as recp,
            tc.tile_pool(name="ysb", bufs=4) as yp,
            tc.tile_pool(name="dram", bufs=1, space="DRAM") as dram,
        ):
            # ---- constants / weights into SBUF ----
            # fp32 biases travel as bf16 hi/lo halves inside the bf16 pack;
            # recombine exactly: f32 = hi + lo (lo = f32 - f32(hi))
            bias_hl = const.tile([P, 32], bf16, tag="bias_hl")
            nc.sync.dma_start(out=bias_hl[:], in_=dview(OFF_BIAS, P, 32))
            bias_sb = const.tile([P, 16], f32, tag="bias")
            nc.vector.tensor_add(bias_sb[:], bias_hl[:, 0:16], bias_hl[:, 16:32])
            bqk_sb = bias_sb[:, 0:8]
            bo_sb = bias_sb[:, 8:16]
            ones_sb = const.tile([1, S], bf16, tag="ones")
            nc.vector.memset(ones_sb[:], 1.0)

            wv_sb = [win.tile([P, CS], bf16, tag=f"wv{k}", name=f"wv{k}") for k in range(KT)]
            for k in range(KT):
                nc.sync.dma_start(out=wv_sb[k][:], in_=dview(OFF_WV + k * P * CS, P, CS))
            wvb_sb = win.tile([1, CS], bf16, tag="wvb")
            nc.sync.dma_start(out=wvb_sb[:], in_=dview(OFF_WV + E * CS, 1, CS))

            xt_sb = [win.tile([P, S], bf16, tag=f"xt{k}", name=f"xt{k}") for k in range(KT)]
            for k in range(KT):
                nc.sync.dma_start(out=xt_sb[k][:], in_=dview(OFF_XT + k * P * S, P, S))

            wqk_sb = [win.tile([P, 2 * CS], bf16, tag=f"wqk{k}", name=f"wqk{k}") for k in range(KT)]
            for k in range(KT):
                nc.sync.dma_start(
                    out=wqk_sb[k][:], in_=dview(OFF_WQK + k * P * 2 * CS, P, 2 * CS)
                )

            wo_sb = [win.tile([P, E], bf16, tag=f"wo{p}", name=f"wo{p}") for p in range(PAIRS)]
            for p in range(PAIRS):
                nc.sync.dma_start(out=wo_sb[p][:], in_=dview(OFF_WO + p * P * E, P, E))

            # persistent activations
            qk_sb = [qkp.tile([P, S], bf16, tag=f"qk{c}", name=f"qk{c}") for c in range(2 * CS // P)]
            v_sb = [vp.tile([P, HPC, D + 1], bf16, tag=f"v{s}", name=f"v{s}") for s in range(SB)]
            on_sb = [onp.tile([P, S], bf16, tag=f"on{p}", name=f"on{p}") for p in range(PAIRS)]

            # ---- shared PSUM pools (8 banks static across all phases) ----
            work_cm = tc.tile_pool(name="work", bufs=2, space="PSUM")
            work = work_cm.__enter__()
            acc_cm = tc.tile_pool(name="acc", bufs=2, space="PSUM")
            acc = acc_cm.__enter__()

            def v_chain(s):
                # V: out[seq-block, vcols] ; lhsT = x^T tile, rhs = Wv tile
                def chain():
                    pv = work.tile([P, CS], f32, tag="w", name=f"pv{s}")
                    for k in range(KT):
                        nc.tensor.matmul(
                            pv[:],
                            xt_sb[k][:, s * P:(s + 1) * P],
                            wv_sb[k][:],
                            start=(k == 0),
                            stop=False,
                        )
                    # bias row: ones row (K=1) x Wv bias row
                    nc.tensor.matmul(
                        pv[:],
                        ones_sb[:, s * P:(s + 1) * P],
                        wvb_sb[:],
                        start=False,
                        stop=True,
                    )
                    nc.vector.memset(v_sb[s][:, :, D:D + 1], 1.0)
                    nc.vector.tensor_copy(v_sb[s][:, :, 0:D], pv[:])
                return chain

            def v_chains():
                return [v_chain(s) for s in range(SB)]

            def qk_chain(c, q):
                def chain():
                    pq = work.tile([P, 512], f32, tag="w", name=f"pq{c}_{q}")
                    for k in range(KT):
                        nc.tensor.matmul(
                            pq[:],
                            wqk_sb[k][:, c * P:(c + 1) * P],
                            xt_sb[k][:, q * 512:(q + 1) * 512],
                            start=(k == 0),
                            stop=(k == KT - 1),
                        )
                    nc.vector.tensor_scalar_add(
                        qk_sb[c][:, q * 512:(q + 1) * 512],
                        pq[:],
                        bqk_sb[:, c:c + 1],
                    )
                return chain

            def qk_pair_chains(p):
                return [qk_chain(c, q) for c in (p, PAIRS + p) for q in range(SQ)]

            def emit_qk_pair(p):
                for ch in qk_pair_chains(p):
                    ch()

            def emit_head(h, fillers=(), stride=2):
                fillers = list(fillers)
                p, half = h // 2, h % 2
                r0 = half * D
                qT = qk_sb[p]
                kT = qk_sb[PAIRS + p]
                po = [
                    acc.tile([D + 1, 1024], f32, tag="a", name=f"po{h}_{j}")
                    for j in range(2)
                ]
                for kb in range(SB):
                    if fillers and kb % stride == stride - 1:
                        fillers.pop(0)()
                    pt = ptp.tile([P, S], bf16, tag="pt", name=f"pt{h}_{kb}")
                    for qh in range(2):
                        ps = work.tile([P, 1024], f32, tag="w", name=f"ps{h}_{kb}_{qh}")
                        for q2 in range(2):
                            q = 2 * qh + q2
                            nc.tensor.matmul(
                                ps[:, q2 * 512:(q2 + 1) * 512],
                                kT[r0:r0 + D, kb * P:(kb + 1) * P],
                                qT[r0:r0 + D, q * 512:(q + 1) * 512],
                                start=True,
                                stop=True,
                            )
                        nc.scalar.activation(
                            pt[:, qh * 1024:(qh + 1) * 1024],
                            ps[:],
                            mybir.ActivationFunctionType.Exp,
                            scale=0.125,
                        )
                        for q2 in range(2):
                            o = qh * 1024 + q2 * 512
                            nc.tensor.matmul(
                                po[qh][:, q2 * 512:(q2 + 1) * 512],
                                v_sb[kb][:, h, :],
                                pt[:, o:o + 512],
                                start=(kb == 0),
                                stop=(kb == SB - 1),
                            )
                # normalize: O^T[0:D] / l (l = row D), write bf16
                for qh in range(2):
                    lsb = recp.tile([1, 1024], f32, tag="lsb", name=f"lsb{h}_{qh}")
                    nc.vector.reciprocal(lsb[:], po[qh][D:D + 1, :])
                    lscr = dram.tile(
                        [1, 1024], f32, tag="lscr", name=f"lscr{h}_{qh}", bufs=2
                    )
                    nc.sync.dma_start(out=lscr[:], in_=lsb[:])
                    ldr = lscr[:]
                    lbc = bass.AP(
                        tensor=ldr.tensor,
                        offset=ldr.offset,
                        ap=[[0, D]] + [list(x) for x in ldr.ap[1:]],
                    )
                    rin = recp.tile([D, 1024], f32, tag="rin", name=f"rin{h}_{qh}")
                    nc.sync.dma_start(out=rin[:], in_=lbc)
                    nc.vector.tensor_mul(
                        on_sb[p][r0:r0 + D, qh * 1024:(qh + 1) * 1024],
                        po[qh][0:D, :],
                        rin[:],
                    )

            # interleave: QK pair 0 first so head 0's S-matmuls (and exp)
            # start ASAP; V is emitted after head 0 so its chains fill PE
            # whenever head 0 blocks (head 0's O-phase pulls v_sb[kb] just in
            # time).  Later QK pairs ride between heads.
            emit_v_phase()
            emit_qk_pair(0)
            emit_head(0)
            emit_head(1)
            emit_qk_pair(1)
            emit_head(2)
            emit_head(3)
            emit_qk_pair(2)
            emit_head(4)
            emit_head(5)
            emit_qk_pair(3)
            emit_head(6)
            emit_head(7)

            # ---- phase 3: output projection + reduce-scatter ----
            # chunk-major layout: each sequence chunk is a contiguous block
            # (collective inputs must be contiguous)
            y_dram = dram.tile([SQ, E, 512], f32, tag="ydram")
            y_red = dram.tile([SQ, E // TP, 512], f32, tag="yred")
            for q in range(SQ):
                for e in range(E // P):
                    py = work.tile([P, 512], f32, tag="w", name=f"py{e}_{q}")
                    for p in range(PAIRS):
                        nc.tensor.matmul(
                            py[:],
                            wo_sb[p][:, e * P:(e + 1) * P],
                            on_sb[p][:, q * 512:(q + 1) * 512],
                            start=(p == 0),
                            stop=(p == PAIRS - 1),
                        )
                    ye = yp.tile([P, 512], f32, tag="ysb")
                    nc.vector.tensor_scalar_add(ye[:], py[:], bo_sb[:, e:e + 1])
                    nc.sync.dma_start(
                        out=y_dram[q, e * P:(e + 1) * P, :],
                        in_=ye[:],
                    )
                # reduce-scatter this sequence chunk while the next computes
                if collective:
                    nc.gpsimd.collective_compute(
                        "ReduceScatter",
                        mybir.AluOpType.add,
                        replica_groups=groups,
                        ins=[y_dram[q].opt()],
                        outs=[y_red[q].opt()],
                    )
                    nc.sync.dma_start(
                        out=out_d[:, q * 512:(q + 1) * 512],
                        in_=y_red[q],
                    )
                else:
                    nc.sync.dma_start(
                        out=out_d[:, q * 512:(q + 1) * 512],
                        in_=y_dram[q, 0:E // TP, :],
                    )

            acc_cm.__exit__(None, None, None)
            work_cm.__exit__(None, None, None)

    from concourse import mybir as _mybir
    _split_drain_waits(nc, _mybir)
    return nc


def _host_shards(x, Wqkv, bqkv, Wo, bo):
    x = np.asarray(x, np.float32)
    Wqkv = np.asarray(Wqkv, np.float32)
    bqkv = np.asarray(bqkv, np.float32)
    Wo = np.asarray(Wo, np.float32)
    bo = np.asarray(bo, np.float32)

    in_maps = []
    for c in range(NCORES):
        b, tp = c // 2, c % 2
        lo = tp * CS
        data = np.empty(NDATA, _BF16)
        data[OFF_XT:OFF_XT + E * S] = x[b].T.astype(_BF16).ravel()
        data[OFF_WQK:OFF_WQK + E * 2 * CS] = (
            np.concatenate([Wqkv[:, lo:lo + CS], Wqkv[:, E + lo:E + lo + CS]], axis=1)
            .astype(_BF16)
            .ravel()
        )
        data[OFF_WV:OFF_WV + (E + 1) * CS] = (
            np.concatenate(
                [
                    Wqkv[:, 2 * E + lo:2 * E + lo + CS],
                    bqkv[None, 2 * E + lo:2 * E + lo + CS],
                ],
                axis=0,
            )
            .astype(_BF16)
            .ravel()
        )
        data[OFF_WO:OFF_WO + CS * E] = Wo[lo:lo + CS, :].astype(_BF16).ravel()

        bqk = (
            np.concatenate([bqkv[lo:lo + CS], bqkv[E + lo:E + lo + CS]])
            .reshape(2 * CS // P, P)
            .T.astype(np.float32)
        )
        bo_c = bo if tp == 0 else np.zeros_like(bo)
        bo_c = bo_c.reshape(E // P, P).T.astype(np.float32)
        bias = np.concatenate([bqk, bo_c], axis=1)
        hi = bias.astype(_BF16)
        lo_ = (bias - hi.astype(np.float32)).astype(_BF16)
        data[OFF_BIAS:NDATA] = np.concatenate([hi, lo_], axis=1).ravel()
        in_maps.append({"data": data})
    return in_maps


def _get_runner():
    """Build the Bass program once and wrap it in a cached 8-core jitted
    callable (same execution path run_bass_kernel_spmd uses under axon, but
    the XLA executable is reused across kernel() calls)."""
    if "runner" in _cached:
        return _cached["runner"]

    import jax
    from jax.sharding import Mesh, PartitionSpec, NamedSharding
    from jax.experimental.shard_map import shard_map
    from concourse import bass2jax, mybir

    nc = _build_program()
    _cached["nc"] = nc
    bass2jax.install_neuronx_cc_hook()

    partition_name = nc.partition_id_tensor.name if nc.partition_id_tensor else None
    in_names, out_names, out_avals = [], [], []
    for alloc in nc.m.functions[0].allocations:
        if not isinstance(alloc, mybir.MemoryLocationSet):
            continue
        name = alloc.memorylocations[0].name
        if alloc.kind == "ExternalInput":
            if name != partition_name:
                in_names.append(name)
        elif alloc.kind == "ExternalOutput":
            out_names.append(name)
            out_avals.append(
                jax.core.ShapedArray(tuple(alloc.tensor_shape), mybir.dt.np(alloc.dtype))
            )
    n_params = len(in_names)
    all_in_names = list(in_names) + list(out_names)
    if partition_name is not None:
        all_in_names.append(partition_name)

    def _body(*args):
        operands = list(args)
        if partition_name is not None:
            operands.append(bass2jax.partition_id_tensor())
        outs = bass2jax._bass_exec_p.bind(
            *operands,
            out_avals=tuple(out_avals),
            in_names=tuple(all_in_names),
            out_names=tuple(out_names),
            lowering_input_output_aliases=(),
            sim_require_finite=True,
            sim_require_nnan=True,
            nc=nc,
        )
        return tuple(outs)

    devices = jax.devices()[:NCORES]
    mesh = Mesh(np.asarray(devices), ("core",))
    in_specs = (PartitionSpec("core"),) * (n_params + len(out_names))
    out_specs = (PartitionSpec("core"),) * len(out_names)
    jitted = jax.jit(
        shard_map(
            _body, mesh=mesh, in_specs=in_specs, out_specs=out_specs, check_rep=False
        ),
        keep_unused=True,
    )
    sharding = NamedSharding(mesh, PartitionSpec("core"))
    zero_shapes = [
        ((NCORES * a.shape[0],) + tuple(a.shape[1:]), a.dtype) for a in out_avals
    ]

    def run(in_maps):
        concat_in = [
            np.concatenate([np.asarray(in_maps[c][nm]) for c in range(NCORES)], axis=0)
            for nm in in_names
        ]
        args = [jax.device_put(a, sharding) for a in concat_in] + [
            jax.device_put(np.zeros(shp, dt), sharding) for shp, dt in zero_shapes
        ]
        outs = jitted(*args)
        outs = [np.asarray(o) for o in outs]
        per_core = [
            {
                nm: outs[i].reshape(NCORES, *out_avals[i].shape)[c]
                for i, nm in enumerate(out_names)
            }
            for c in range(NCORES)
        ]
        return per_core

    _cached["runner"] = run
    _cached["jitted"] = jitted
    _cached["meta"] = (in_names, out_names, out_avals, sharding)
    return run


def kernel(x, Wqkv, bqkv, Wo, bo):
    run = _get_runner()
    in_maps = _host_shards(x, Wqkv, bqkv, Wo, bo)
    results = run(in_maps)

    out = np.empty((B, S, E), np.float32)
    for b in range(B):
        yT = np.concatenate(
            [results[2 * b]["out"], results[2 * b + 1]["out"]], axis=0
        )
        out[b] = yT.T
    return out
